# revision 2
# baseline (speedup 1.0000x reference)
"""GatedAttention Trainium2 kernel, 8-way tensor-parallel over heads.

Reference computation (B=1, S=2048, D=2048, H=16 heads, Hd=128):
  q,k,v = x @ {q,k,v}_w.T  (per-head split)
  scores = (q @ k.T) / sqrt(Hd), causal mask, softmax
  av = attn @ v
  gate = sigmoid(q @ gate_w.T + gate_b)       (per-head)
  y = concat_heads(av * gate) @ o_w.T

Sharding: 2 heads per core (column-parallel QKV/gate), AllGather of the
gated per-head outputs (bf16, transposed [feature, seq] layout), then
column-parallel o_proj; host concatenates the 8 output column slices.

All matmuls run on the PE in bf16 with fp32 PSUM accumulation. Softmax is
computed without max-subtraction (scores are small by construction);
row sums of exp ride on the PE as M=1 ones-matmuls in the same [j, q]
transposed layout, so no on-chip transposes are needed anywhere.
"""

import numpy as np
import ml_dtypes

import concourse.bass as bass
import concourse.mybir as mybir
import concourse.tile as tile
from concourse import bacc
from concourse.bass_utils import run_bass_kernel_spmd

BF16 = ml_dtypes.bfloat16
F32 = mybir.dt.float32
BF = mybir.dt.bfloat16

N_CORES = 8
S = 2048          # sequence length
D = 2048          # model dim
H = 16            # total heads
HD = 128          # head dim
HEADS_PER_CORE = H // N_CORES        # 2
E = HEADS_PER_CORE * HD              # 256 output dims per core
DC = D // 128                        # 16 contraction chunks
QCW = 512                            # q-chunk width
NQC = S // QCW                       # 4 q-chunks
SCALE = 1.0 / float(np.sqrt(HD))

_CACHED = {}


def _build():
    nc = bacc.Bacc("TRN2", target_bir_lowering=False, debug=False,
                   num_devices=N_CORES, enable_asserts=False)

    xt = nc.dram_tensor("xt", [D, S], BF, kind="ExternalInput")        # x^T
    wqt = nc.dram_tensor("wqt", [D, E], BF, kind="ExternalInput")      # q_w shard^T
    wkt = nc.dram_tensor("wkt", [D, E], BF, kind="ExternalInput")
    wvt = nc.dram_tensor("wvt", [D, E], BF, kind="ExternalInput")
    owt = nc.dram_tensor("owt", [D, E], BF, kind="ExternalInput")      # o_w shard^T
    gwt = nc.dram_tensor("gwt", [HD, HD], BF, kind="ExternalInput")    # gate_w^T
    gb = nc.dram_tensor("gb", [HD, 1], F32, kind="ExternalInput")      # gate bias
    trim = nc.dram_tensor("trim", [128, 128], BF, kind="ExternalInput")  # causal triangle
    yt = nc.dram_tensor("yt", [E, S], F32, kind="ExternalOutput")      # y^T col-slice

    with tile.TileContext(nc) as tc:
        with tc.tile_pool(name="const", bufs=1) as const, \
             tc.tile_pool(name="work", bufs=2) as work, \
             tc.tile_pool(name="psum", bufs=1, space="PSUM") as psum, \
             tc.tile_pool(name="dram", bufs=1, space="DRAM") as dram:

            # ---- constant / resident loads ----
            xts = const.tile([128, DC, S], BF, tag="xts")
            wqts = const.tile([128, DC, E], BF, tag="wqts")
            wkts = const.tile([128, DC, E], BF, tag="wkts")
            wvts = const.tile([128, DC, E], BF, tag="wvts")
            owts = const.tile([128, DC, E], BF, tag="owts")
            gwts = const.tile([HD, HD], BF, tag="gwts")
            gbs = const.tile([HD, 1], F32, tag="gbs")
            tris = const.tile([128, 128], BF, tag="tris")
            ones128 = const.tile([128, 1], BF, tag="ones128")
            one1 = const.tile([1, 128], F32, tag="one1")

            for dc in range(DC):
                nc.sync.dma_start(wqts[:, dc, :], wqt.ap()[dc * 128:(dc + 1) * 128, :])
                nc.sync.dma_start(wkts[:, dc, :], wkt.ap()[dc * 128:(dc + 1) * 128, :])
                nc.sync.dma_start(wvts[:, dc, :], wvt.ap()[dc * 128:(dc + 1) * 128, :])
                nc.sync.dma_start(xts[:, dc, :], xt.ap()[dc * 128:(dc + 1) * 128, :])
                nc.sync.dma_start(owts[:, dc, :], owt.ap()[dc * 128:(dc + 1) * 128, :])
            nc.sync.dma_start(gwts[:], gwt.ap())
            nc.sync.dma_start(gbs[:], gb.ap())
            nc.sync.dma_start(tris[:], trim.ap())
            nc.vector.memset(ones128[:], 1.0)
            nc.vector.memset(one1[:], 1.0)

            # ---- projections ----
            # Q^T, K^T: [e(2x128), s] layout.  V: [s(16x128), e] layout.
            qts = const.tile([128, HEADS_PER_CORE, S], BF, tag="qts")
            kts = const.tile([128, HEADS_PER_CORE, S], BF, tag="kts")
            vts = const.tile([128, DC, E], BF, tag="vts")

            for (wts, outts) in ((wqts, qts), (wkts, kts)):
                for ec in range(HEADS_PER_CORE):
                    for sc in range(NQC):
                        pp = psum.tile([128, QCW], F32, tag="mmbig", bufs=3, name="pp")
                        for dc in range(DC):
                            nc.tensor.matmul(
                                pp[:],
                                wts[:, dc, ec * 128:(ec + 1) * 128],
                                xts[:, dc, sc * QCW:(sc + 1) * QCW],
                                start=(dc == 0), stop=(dc == DC - 1))
                        nc.vector.tensor_copy(
                            out=outts[:, ec, sc * QCW:(sc + 1) * QCW], in_=pp[:])

            for sc16 in range(DC):   # 16 chunks of 128 seq rows
                vp = psum.tile([128, QCW], F32, tag="mmbig", bufs=3, name="vp")
                for dc in range(DC):
                    nc.tensor.matmul(
                        vp[:, :E],
                        xts[:, dc, sc16 * 128:(sc16 + 1) * 128],
                        wvts[:, dc, :],
                        start=(dc == 0), stop=(dc == DC - 1))
                nc.vector.tensor_copy(out=vts[:, sc16, :], in_=vp[:, :E])

            # ---- attention (per head, transposed layout) ----
            attd = dram.tile([E, S], BF, tag="attd")  # gated output^T, AG input

            for h in range(HEADS_PER_CORE):
                for qc in range(NQC):
                    q0 = qc * QCW
                    # gate^T = sigmoid(gate_w @ Q_h^T + b)
                    gp = psum.tile([128, QCW], F32, tag="mmbig", bufs=3, name="gp")
                    nc.tensor.matmul(gp[:], gwts[:], qts[:, h, q0:q0 + QCW],
                                     start=True, stop=True)
                    gs = work.tile([128, QCW], F32, tag="gs", bufs=2, name="gs")
                    nc.scalar.activation(gs[:], gp[:],
                                         mybir.ActivationFunctionType.Sigmoid,
                                         bias=gbs[:, 0:1])

                    avp = psum.tile([128, QCW], F32, tag="avp", bufs=1, name="avp")
                    sump = psum.tile([1, QCW], F32, tag="sump", bufs=1, name="sump")
                    njj = 4 * qc + 4
                    for jj in range(njj):
                        off = jj - 4 * qc
                        s0 = max(0, off * 128)
                        scp = psum.tile([128, QCW], F32, tag="scp", bufs=2, name="scp")
                        nc.tensor.matmul(
                            scp[:, s0:], kts[:, h, jj * 128:(jj + 1) * 128],
                            qts[:, h, q0 + s0:q0 + QCW], start=True, stop=True)
                        ext = work.tile([128, QCW], BF, tag="ext", bufs=3, name="ext")
                        nc.scalar.activation(ext[:, s0:], scp[:, s0:],
                                             mybir.ActivationFunctionType.Exp,
                                             scale=SCALE)
                        if off >= 0:
                            nc.vector.tensor_mul(ext[:, s0:s0 + 128],
                                                 ext[:, s0:s0 + 128], tris[:])
                        nc.tensor.matmul(
                            avp[:, s0:], vts[:, jj, h * 128:(h + 1) * 128],
                            ext[:, s0:],
                            start=(jj == 0), stop=(jj == njj - 1))
                        nc.tensor.matmul(
                            sump[:, s0:], ones128[:], ext[:, s0:],
                            start=(jj == 0), stop=(jj == njj - 1))

                    rs = work.tile([1, QCW], F32, tag="rs", bufs=2, name="rs")
                    nc.vector.reciprocal(out=rs[:], in_=sump[:])
                    bcp = psum.tile([128, QCW], F32, tag="mmbig", bufs=3, name="bcp")
                    nc.tensor.matmul(bcp[:], one1[:], rs[:], start=True, stop=True)
                    gn = work.tile([128, QCW], F32, tag="gn", bufs=2, name="gn")
                    nc.vector.tensor_mul(gn[:], gs[:], bcp[:])
                    att = work.tile([128, QCW], BF, tag="att", bufs=2, name="att")
                    nc.vector.tensor_mul(att[:], avp[:], gn[:])
                    nc.sync.dma_start(attd[h * 128:(h + 1) * 128, q0:q0 + QCW], att[:])

            # ---- AllGather gated outputs: [E, S] x 8 -> [D, S] ----
            outd = dram.tile([N_CORES * E, S], BF, tag="outd", addr_space="Shared")
            nc.gpsimd.collective_compute(
                "AllGather", mybir.AluOpType.bypass,
                replica_groups=[list(range(N_CORES))],
                ins=[attd[:].opt()], outs=[outd[:].opt()])

            # ---- o_proj: y^T[e', s] = sum_f o_w[cs+e', f] out^T[f, s] ----
            for sc in range(NQC):
                yps = [psum.tile([128, QCW], F32, tag="mmbig", bufs=3, name="yp")
                       for _ in range(HEADS_PER_CORE)]
                for fc in range(DC):
                    ot = work.tile([128, QCW], BF, tag="ot", bufs=4, name="ot")
                    nc.sync.dma_start(
                        ot[:], outd[fc * 128:(fc + 1) * 128, sc * QCW:(sc + 1) * QCW])
                    for ec in range(HEADS_PER_CORE):
                        nc.tensor.matmul(
                            yps[ec][:], owts[:, fc, ec * 128:(ec + 1) * 128], ot[:],
                            start=(fc == 0), stop=(fc == DC - 1))
                for ec in range(HEADS_PER_CORE):
                    ys = work.tile([128, QCW], F32, tag="ys", bufs=2, name="ys")
                    nc.vector.tensor_copy(out=ys[:], in_=yps[ec][:])
                    nc.sync.dma_start(
                        yt.ap()[ec * 128:(ec + 1) * 128, sc * QCW:(sc + 1) * QCW],
                        ys[:])

    nc.compile()
    return nc


def _prep_inputs(x, q_w, k_w, v_w, o_w, gate_w, gate_b):
    x = np.asarray(x, dtype=np.float32)
    xt = np.ascontiguousarray(x.reshape(S, D).T).astype(BF16)
    gwt = np.ascontiguousarray(np.asarray(gate_w, np.float32).T).astype(BF16)
    gb = np.asarray(gate_b, np.float32).reshape(HD, 1).copy()
    trim = np.triu(np.ones((128, 128), np.float32)).astype(BF16)
    in_maps = []
    for c in range(N_CORES):
        sl = slice(c * E, (c + 1) * E)
        in_maps.append({
            "xt": xt,
            "wqt": np.ascontiguousarray(np.asarray(q_w, np.float32)[sl, :].T).astype(BF16),
            "wkt": np.ascontiguousarray(np.asarray(k_w, np.float32)[sl, :].T).astype(BF16),
            "wvt": np.ascontiguousarray(np.asarray(v_w, np.float32)[sl, :].T).astype(BF16),
            "owt": np.ascontiguousarray(np.asarray(o_w, np.float32)[sl, :].T).astype(BF16),
            "gwt": gwt,
            "gb": gb,
            "trim": trim,
        })
    return in_maps


def _run(in_maps, **kwargs):
    if "nc" not in _CACHED:
        _CACHED["nc"] = _build()
    return run_bass_kernel_spmd(_CACHED["nc"], in_maps,
                                core_ids=list(range(N_CORES)), **kwargs)


def kernel(x, q_w, k_w, v_w, o_w, gate_w, gate_b):
    res = _run(_prep_inputs(x, q_w, k_w, v_w, o_w, gate_w, gate_b))
    yts = [res.results[c]["yt"] for c in range(N_CORES)]
    y_t = np.concatenate(yts, axis=0)          # [D(e), S]
    return np.ascontiguousarray(y_t.T, dtype=np.float32).reshape(1, S, D)
